# revision 1
# baseline (speedup 1.0000x reference)
"""Multi-head causal attention (B=4, T=2048, C=1024, H=16) on 8 TRN2 cores.

Sharding: core i handles batch b = i//2 and head-group g = i%2 (8 heads each).
Each core computes qkv projection for its heads, causal attention, and a
partial output projection (its heads' rows of W_o). The host sums the two
partials per batch and adds b_o.

Device kernel (per core, same SPMD program), all matmuls bf16 with fp32 PSUM,
fully software-pipelined by head pair so the qkv projection of head-pair
hp+1 fills the PE while the exp-paced attention of head-pair hp runs:

  - qkT = (Wqk^T x^T) transposed: [1024 feats, 2048] bf16, emitted per
    128-feature tile as pipeline filler
  - v   = x Wv natural: [2048, 512] bf16, per 128-token tile as filler
  - attention per head pair (2hp, 2hp+1), hp-major, per 512-query block tb,
    per 128-key chunk j (causal-trimmed):
      S^T = K^T q^T  [128 tk, tq]  (two heads at PE row groups 0-63/64-127,
                                    concurrent on HW), fp32 PSUM [128,1024]
      P^T = exp(S^T * 0.125)       (ScalarE; diagonal chunks masked on DVE)
      AV^T += [V | 1]^T P^T        [65, tq]  (row 64 = softmax denominator)
    S chunks of segment s+1 interleave with AV chunks of segment s.
    AV drains: even head on DVE, odd head on ACT (concurrent), sums on DVE.
  - softmax normalization per segment (lagged one segment): reciprocal on
    DVE, broadcast across partitions via ones-outer matmul into PSUM,
    single in-place DVE multiply of attn from the PSUM operand
  - out_part = attT^T W_o rows [2048, 1024]: per 128-token tile as stage-3
    filler as soon as its last segment normalizes; host sums the two
    per-batch partials in fp32 and adds b_o.

PSUM budget (8 banks): scores [128,1024]x2 = 4, AV [.,512]x2 = 2,
aux (proj/norm-broadcast/oproj) [128,512]x2 = 2.
"""

import sys

sys.path.insert(0, "/opt/trn_rl_repo")

import numpy as np
import ml_dtypes

BF16 = ml_dtypes.bfloat16

B, T, C, H, D = 4, 2048, 1024, 16, 64
HPC = 8        # heads per core
CQ = HPC * D   # 512
NCORES = 8
P = 128
NTT = T // 512  # 4 query blocks
VW = HPC * 65   # 520: v row layout (64 cols + ones col per head)


def _split_waits(nc):
    """This container's walrus accepts only ONE sync wait per instruction.
    Split any instruction carrying N>1 waits into N-1 single-wait NoOps on
    the same engine immediately before it."""
    import concourse.mybir as mybir

    ctr = 0
    for fn in nc.m.functions:
        for bb in fn.blocks:
            insts = list(bb.instructions)
            new_insts = []
            changed = False
            for inst in insts:
                si = inst.sync_info
                if si is not None and si.on_wait and len(si.on_wait) > 1:
                    waits = list(si.on_wait)
                    for w in waits[:-1]:
                        ctr += 1
                        nop = mybir.InstNoOp(
                            name=f"I-wsplit-{ctr}",
                            engine=inst.engine,
                            ins=[],
                            outs=[],
                            sync_info=mybir.SyncInfo(on_wait=[w], on_update=[]),
                        )
                        new_insts.append(nop)
                    si.on_wait = [waits[-1]]
                    changed = True
                new_insts.append(inst)
            if changed:
                bb.instructions[:] = new_insts
    return ctr


def _declare(nc):
    import concourse.mybir as mybir

    bf = mybir.dt.bfloat16
    f32 = mybir.dt.float32
    return dict(
        xT=nc.dram_tensor("xT", [C, T], bf, kind="ExternalInput").ap(),
        wqk=nc.dram_tensor("wqk", [C, 2 * CQ], bf, kind="ExternalInput").ap(),
        bqk=nc.dram_tensor("bqk", [P, 8], f32, kind="ExternalInput").ap(),
        wv=nc.dram_tensor("wv", [C, CQ], bf, kind="ExternalInput").ap(),
        bvb=nc.dram_tensor("bvb", [P, CQ], f32, kind="ExternalInput").ap(),
        wo=nc.dram_tensor("wo", [CQ, C], bf, kind="ExternalInput").ap(),
        maskT=nc.dram_tensor("maskT", [P, P], bf, kind="ExternalInput").ap(),
        outp=nc.dram_tensor("outp", [T, C], bf, kind="ExternalOutput").ap(),
    )


def _emit(nc, tc, aps):
    import concourse.mybir as mybir
    from concourse.alu_op_type import AluOpType

    bf = mybir.dt.bfloat16
    f32 = mybir.dt.float32
    Exp = mybir.ActivationFunctionType.Exp

    xT = aps["xT"]; wqk = aps["wqk"]; bqk = aps["bqk"]; wv = aps["wv"]
    bvb = aps["bvb"]; wo = aps["wo"]; maskT = aps["maskT"]; outp = aps["outp"]

    with tc.tile_pool(name="const", bufs=1) as cpool:
        bqk_sb = cpool.tile([P, 8], f32)
        bvb_sb = cpool.tile([P, CQ], f32)
        maskT_sb = cpool.tile([P, P], bf)
        ones1_sb = cpool.tile([1, 64], bf)
        xT_sb = cpool.tile([P, 8 * T], bf)
        wqk_sb = cpool.tile([P, 8 * 1024], bf)
        wv_sb = cpool.tile([P, 8 * CQ], bf)
        wo_sb = cpool.tile([P, 4 * 1024], bf)
        qkT_sb = cpool.tile([P, 8 * T], bf)
        v_sb = cpool.tile([P, 16 * VW], bf)
        attn_sb = cpool.tile([P, 16 * 512], bf)

        # DMAs in consumption order: the wqk/xT quarters the prologue
        # matmuls stream first (in cc order), small constants next, then
        # wv, the later xT quarters, and wo (only needed by stage 3).
        for cc in range(8):
            nc.sync.dma_start(wqk_sb[:, cc * 1024:(cc + 1) * 1024],
                              wqk[cc * P:(cc + 1) * P, :])
            nc.sync.dma_start(xT_sb[:, cc * T: cc * T + 512],
                              xT[cc * P:(cc + 1) * P, 0:512])
        nc.sync.dma_start(bqk_sb[:], bqk[:])
        nc.sync.dma_start(bvb_sb[:], bvb[:])
        nc.sync.dma_start(maskT_sb[:], maskT[:])
        for cc in range(8):
            nc.sync.dma_start(wv_sb[:, cc * CQ:(cc + 1) * CQ],
                              wv[cc * P:(cc + 1) * P, :])
        for cc in range(8):
            nc.sync.dma_start(xT_sb[:, cc * T + 512: cc * T + 1024],
                              xT[cc * P:(cc + 1) * P, 512:1024])
        for cc in range(8):
            nc.sync.dma_start(xT_sb[:, cc * T + 1024: (cc + 1) * T],
                              xT[cc * P:(cc + 1) * P, 1024:T])
        for hc in range(4):
            nc.sync.dma_start(wo_sb[:, hc * 1024:(hc + 1) * 1024],
                              wo[hc * P:(hc + 1) * P, :])

        nc.vector.memset(ones1_sb[:], 1.0)
        v_ones = v_sb.rearrange("p (a c) -> p a c", c=65)
        nc.vector.memset(v_ones[:, :, 64:65], 1.0)

        with tc.tile_pool(name="ps_s", bufs=1, space="PSUM") as ps_s, \
             tc.tile_pool(name="ps_av", bufs=1, space="PSUM") as ps_av, \
             tc.tile_pool(name="ps_aux", bufs=1, space="PSUM") as ps_aux, \
             tc.tile_pool(name="work", bufs=1) as wpool:

            pts = {}
            sums = {}

            # ---------- pipeline building blocks ----------
            def proj_qk(nt, tt):
                # qkT feature tile nt (q: nt=hp, k: nt=4+hp), 512 tokens
                psq = ps_aux.tile([P, 512], f32, tag="aux", bufs=2,
                                  name=f"psq_{nt}_{tt}")
                for cc in range(8):
                    nc.tensor.matmul(
                        psq[:],
                        wqk_sb[:, cc * 1024 + nt * P: cc * 1024 + (nt + 1) * P],
                        xT_sb[:, cc * T + tt * 512: cc * T + (tt + 1) * 512],
                        start=(cc == 0), stop=(cc == 7),
                    )
                nc.vector.tensor_scalar(
                    qkT_sb[:, nt * T + tt * 512: nt * T + (tt + 1) * 512],
                    psq[:], bqk_sb[:, nt:nt + 1], None, op0=AluOpType.add,
                )

            def proj_v(t16):
                psv = ps_aux.tile([P, CQ], f32, tag="aux", bufs=2,
                                  name=f"psv_{t16}")
                for cc in range(8):
                    nc.tensor.matmul(
                        psv[:],
                        xT_sb[:, cc * T + t16 * P: cc * T + (t16 + 1) * P],
                        wv_sb[:, cc * CQ:(cc + 1) * CQ],
                        start=(cc == 0), stop=(cc == 7),
                    )
                vv = v_sb[:, t16 * VW:(t16 + 1) * VW].rearrange(
                    "p (a c) -> p a c", c=65)
                nc.vector.tensor_tensor(
                    vv[:, :, 0:64],
                    psv[:].rearrange("p (a c) -> p a c", c=64),
                    bvb_sb[:].rearrange("p (a c) -> p a c", c=64),
                    op=AluOpType.add,
                )

            def s_chunk(hp, tb, j):
                h0, h1 = 2 * hp, 2 * hp + 1
                off = j * P - tb * 512
                nstart = max(off, 0)
                pss = ps_s.tile([P, 1024], f32, tag="pss", bufs=2,
                                name=f"pss_{hp}_{tb}_{j}")
                pt = wpool.tile([P, 1024], bf, tag="pt", bufs=18,
                                name=f"pt_{hp}_{tb}_{j}")
                pts[(hp, tb, j)] = pt
                for i, hl in enumerate((h0, h1)):
                    base = (hl % 2) * 64
                    nc.tensor.matmul(
                        pss[:, i * 512 + nstart: i * 512 + 512],
                        qkT_sb[base:base + 64,
                               (4 + hp) * T + j * P: (4 + hp) * T + (j + 1) * P],
                        qkT_sb[base:base + 64,
                               hp * T + tb * 512 + nstart: hp * T + (tb + 1) * 512],
                        start=True, stop=True,
                    )
                pw = pss.rearrange("p (a c) -> p a c", c=512)
                ptw = pt.rearrange("p (a c) -> p a c", c=512)
                nc.scalar.activation(
                    ptw[:, :, nstart:512], pw[:, :, nstart:512], Exp, scale=0.125,
                )
                if off >= 0:
                    # one DVE op masks both heads' diagonal blocks (2D AP
                    # over the two 512-strided regions, mask broadcast)
                    blk = pt.rearrange(
                        "p (a c) -> p a c", c=512)[:, :, nstart:nstart + P]
                    mb = maskT_sb.rearrange(
                        "p (a f) -> p a f", a=1).broadcast_to([P, 2, P])
                    nc.vector.tensor_tensor(blk, blk, mb, op=AluOpType.mult)

            def av_chunk(hp, tb, j, psav):
                h0, h1 = 2 * hp, 2 * hp + 1
                jmax = 4 * tb + 3
                off = j * P - tb * 512
                nstart = max(off, 0)
                for i, hl in enumerate((h0, h1)):
                    nc.tensor.matmul(
                        psav[i][0:65, nstart:512],
                        v_sb[:, j * VW + hl * 65: j * VW + (hl + 1) * 65],
                        pts[(hp, tb, j)][:, i * 512 + nstart: i * 512 + 512],
                        start=(j == 0), stop=(j == jmax),
                    )
                if j == jmax:
                    seg = hp * NTT + tb
                    sl = slice(seg * 512, (seg + 1) * 512)
                    s_e = wpool.tile([1, 512], f32, tag="sums_e", bufs=4,
                                     name=f"sums_e_{seg}")
                    s_o = wpool.tile([1, 512], f32, tag="sums_o", bufs=4,
                                     name=f"sums_o_{seg}")
                    sums[seg] = (s_e, s_o)
                    # even head drains on DVE, odd head on ACT (concurrent),
                    # denominator rows on DVE
                    nc.vector.tensor_copy(attn_sb[0:64, sl], psav[0][0:64, :])
                    nc.scalar.copy(attn_sb[64:128, sl], psav[1][0:64, :])
                    nc.vector.tensor_copy(s_e[:], psav[0][64:65, :])
                    nc.vector.tensor_copy(s_o[:], psav[1][64:65, :])

            def norm_seg(seg):
                sl = slice(seg * 512, (seg + 1) * 512)
                s_e, s_o = sums.pop(seg)
                rec_e = wpool.tile([1, 512], bf, tag="rece", bufs=3,
                                   name=f"rece_{seg}")
                rec_o = wpool.tile([1, 512], bf, tag="reco", bufs=3,
                                   name=f"reco_{seg}")
                with nc.allow_low_precision(reason="bf16 softmax denominators"):
                    nc.vector.reciprocal(rec_e[:], s_e[:])
                    nc.vector.reciprocal(rec_o[:], s_o[:])
                psr = ps_aux.tile([P, 512], f32, tag="aux", bufs=2,
                                  name=f"psr_{seg}")
                nc.tensor.matmul(psr[0:64, :], ones1_sb[:], rec_e[:],
                                 start=True, stop=True)
                nc.tensor.matmul(psr[64:128, :], ones1_sb[:], rec_o[:],
                                 start=True, stop=True)
                nc.vector.tensor_tensor(attn_sb[:, sl], attn_sb[:, sl], psr[:],
                                        op=AluOpType.mult)

            def oproj_tt(tt16, pspool=None, ptag="aux"):
                # epilogue groups draw from the AV pool instead: its banks
                # retire at segment-15's drains, so the NEXT body's first
                # projection (which rotates through the aux pool) need not
                # wait for the very last output tiles here
                psos = [
                    (pspool or ps_aux).tile([P, 512], f32, tag=ptag, bufs=2,
                                            name=f"pso_{tt16}_{mb}")
                    for mb in range(2)
                ]
                for hc in range(4):
                    seg = hc * NTT + tt16 // 4
                    col = (seg * 4 + tt16 % 4) * P
                    for mb in range(2):
                        nc.tensor.matmul(
                            psos[mb][:],
                            attn_sb[:, col: col + P],
                            wo_sb[:, hc * 1024 + mb * 512: hc * 1024 + (mb + 1) * 512],
                            start=(hc == 0), stop=(hc == 3),
                        )
                for mb in range(2):
                    osb = wpool.tile([P, 512], bf, tag="osb", bufs=4,
                                     name=f"osb_{tt16}_{mb}")
                    nc.vector.tensor_copy(osb[:], psos[mb][:])
                    nc.sync.dma_start(
                        outp[tt16 * P:(tt16 + 1) * P, mb * 512:(mb + 1) * 512],
                        osb[:],
                    )

            # ---------- per-segment filler schedules ----------
            # Filler emitted during segment s must only feed emissions of
            # segment s+1 or later (PE queue is in-order; anything a queued
            # instruction waits on must already be in the queue).
            def fillers_for(s):
                hp, tb = s // 4, s % 4
                out = []
                if hp == 0:
                    if tb < 2:
                        out.append(lambda tt=tb + 2: proj_qk(0, tt))
                        out.append(lambda tt=tb + 2: proj_qk(4, tt))
                    if tb < 3:
                        for i in range(4):
                            out.append(lambda t=4 * (tb + 1) + i: proj_v(t))
                if hp < 3:
                    out.append(lambda h=hp + 1, tt=tb: proj_qk(h, tt))
                    out.append(lambda h=hp + 1, tt=tb: proj_qk(4 + h, tt))
                if hp == 3 and tb >= 1:
                    for i in range(4):
                        out.append(lambda t=4 * (tb - 1) + i: oproj_tt(t))
                return out

            # ---------- prologue ----------
            proj_qk(0, 0)   # q features of head pair 0, tokens 0:512
            proj_qk(4, 0)   # k features
            for j in range(4):
                s_chunk(0, 0, j)   # spool up ACT as early as possible
            for t16 in range(4):
                proj_v(t16)
            proj_qk(0, 1)   # q/k for query block 1 (its S chunks are
            proj_qk(4, 1)   # emitted inside segment 0's m-loop)

            # ---------- main pipeline ----------
            for s in range(16):
                hp, tb = s // 4, s % 4
                psav = [
                    ps_av.tile([P, 512], f32, tag="psav", bufs=2,
                               name=f"psav_{hp}_{tb}_{i}")
                    for i in range(2)
                ]
                js_a = list(range(4 * tb + 4))
                if s + 1 < 16:
                    nhp, ntb = (s + 1) // 4, (s + 1) % 4
                    js_s = list(range(4 * ntb + 4))
                else:
                    js_s = []
                fill = fillers_for(s)
                nsteps = max(len(js_a), len(js_s))
                # spread fillers over the m-loop, front-loaded after m=1
                for m in range(nsteps):
                    if s > 0 and m == 1:
                        norm_seg(s - 1)
                    if m < len(js_s):
                        s_chunk(nhp, ntb, js_s[m])
                    if m < len(js_a):
                        av_chunk(hp, tb, js_a[m], psav)
                    while fill and len(fill) >= (nsteps - m):
                        fill.pop(0)()
                while fill:
                    fill.pop(0)()

            # ---------- epilogue ----------
            norm_seg(15)
            for t16 in range(12, 16):
                oproj_tt(t16, pspool=ps_av, ptag="psav")


_cached = {}


def build_program(split=True, ncopies=1):
    key = ("nc", ncopies)
    if key not in _cached:
        import concourse.bass as bass
        import concourse.tile as tile

        nc = bass.Bass("TRN2", target_bir_lowering=False, debug=False)
        with tile.TileContext(nc) as tc:
            aps = _declare(nc)
            for _ in range(ncopies):
                _emit(nc, tc, aps)
        _cached[key] = nc
    if split and not _cached.get(("split", ncopies)):
        _split_waits(_cached[key])
        _cached[("split", ncopies)] = True
    return _cached[key]


def make_in_maps(x, W_qkv, b_qkv, W_o):
    x = np.asarray(x, dtype=np.float32)
    W_qkv = np.asarray(W_qkv, dtype=np.float32)
    b_qkv = np.asarray(b_qkv, dtype=np.float32)
    W_o = np.asarray(W_o, dtype=np.float32)
    maskT = np.triu(np.ones((P, P), np.float32)).astype(BF16)
    in_maps = []
    for core in range(NCORES):
        b, g = core // 2, core % 2
        qs = slice(g * CQ, (g + 1) * CQ)
        xTc = np.ascontiguousarray(x[b].T).astype(BF16)
        wq = W_qkv[:, 0:C][:, qs]
        wk = W_qkv[:, C:2 * C][:, qs]
        wvs = np.ascontiguousarray(W_qkv[:, 2 * C:3 * C][:, qs]).astype(BF16)
        wqks = np.ascontiguousarray(np.concatenate([wq, wk], axis=1)).astype(BF16)
        bq = b_qkv[0:C][qs]
        bk = b_qkv[C:2 * C][qs]
        bv = b_qkv[2 * C:3 * C][qs]
        bqk_t = np.ascontiguousarray(
            np.concatenate([bq, bk]).reshape(8, P).T
        ).astype(np.float32)
        bvb = np.ascontiguousarray(
            np.broadcast_to(bv, (P, CQ))
        ).astype(np.float32)
        wos = np.ascontiguousarray(W_o[qs, :]).astype(BF16)
        in_maps.append(
            dict(xT=xTc, wqk=wqks, bqk=bqk_t, wv=wvs, bvb=bvb, wo=wos,
                 maskT=maskT)
        )
    return in_maps


def run(x, W_qkv, b_qkv, W_o, b_o, trace=False, trace_kwargs=None):
    import time as _time

    from concourse.bass_utils import run_bass_kernel_spmd

    nc = build_program()
    in_maps = make_in_maps(x, W_qkv, b_qkv, W_o)
    last_err = None
    for attempt in range(3):
        try:
            res = run_bass_kernel_spmd(
                nc, in_maps, core_ids=list(range(NCORES)), trace=trace,
                **(trace_kwargs or {}),
            )
            break
        except Exception as e:  # transient device wedge -> retry
            last_err = e
            _time.sleep(5)
    else:
        raise last_err
    b_o = np.asarray(b_o, dtype=np.float32)
    out = np.empty((B, T, C), np.float32)
    for b in range(B):
        out[b] = (res.results[2 * b]["outp"].astype(np.float32)
                  + res.results[2 * b + 1]["outp"].astype(np.float32) + b_o)
    return out, res


def kernel(x, W_qkv, b_qkv, W_o, b_o):
    out, _ = run(x, W_qkv, b_qkv, W_o, b_o, trace=False)
    return out



# revision 15
# speedup vs baseline: 6.4255x; 6.4255x over previous
"""Multi-head causal attention (B=4, T=2048, C=1024, H=16) on 8 TRN2 cores.

Sharding: core i handles batch b = i//2 and head-group g = i%2 (8 heads each).
Each core computes qkv projection for its heads, causal attention, and a
partial output projection (its heads' rows of W_o). The host sums the two
partials per batch and adds b_o.

v2: fp8 (e4m3) DoubleRow matmuls for the qkv projection (tokens >= 512) and
the P@V attention matmul (two 128-key chunks summed per DR pass), with a
bf16 "island" (queries 0:512 x keys 0:256 + all tt=0 projections) protecting
the few-key early tokens whose softmax averaging cannot wash out fp8 noise.
S = Q@K stays bf16 (DoubleRow needs full 128-wide stationary tiles to hit
0.5 cyc/row; the 32-partition S packing runs at 1.0).

Scales (powers of 2, exact): x stored x4, W_qkv/W_v x64 -> proj PSUM = 256x;
q,k stored x1 bf16 ((psq+256b)/256); v stored x8 fp8 ((psv/32)+8b); P x1;
AV PSUM = 8x, normalized by (1/8)/denom via the ones-broadcast matmul.

Causal masking is done on the PE: tiny identity-matmuls accumulate -1e4 into
the S PSUM (strict-lower-triangle tile for diagonal blocks, a full -1e4
block for the below-diagonal strip of odd pair chunks) so exp() emits exact
zeros and the DVE does no mask work.

Per-head AV weights are padded to 128 columns ([64 v | ones | 63 zeros]) so
the DoubleRow matmul uses a full column tile (the fast 0.5 cyc/row path);
PSUM row 64 accumulates the softmax denominator, rows 65:127 are garbage.
"""

import sys

sys.path.insert(0, "/opt/trn_rl_repo")

import numpy as np
import ml_dtypes

BF16 = ml_dtypes.bfloat16
F8E4 = ml_dtypes.float8_e4m3

B, T, C, H, D = 4, 2048, 1024, 16, 64
HPC = 8        # heads per core
CQ = HPC * D   # 512
NCORES = 8
P = 128
NTT = T // 512  # 4 query blocks
VW = HPC * 65   # island v row layout (64 cols + ones col per head)
VW8 = HPC * 128  # fp8 v row layout (64 v + 1 ones + 63 pad per head)


def _split_waits(nc):
    """This container's walrus accepts only ONE sync wait per instruction.
    Split any instruction carrying N>1 waits into N-1 single-wait NoOps on
    the same engine immediately before it."""
    import concourse.mybir as mybir

    ctr = 0
    for fn in nc.m.functions:
        for bb in fn.blocks:
            insts = list(bb.instructions)
            new_insts = []
            changed = False
            for inst in insts:
                si = inst.sync_info
                if si is not None and si.on_wait and len(si.on_wait) > 1:
                    waits = list(si.on_wait)
                    for w in waits[:-1]:
                        ctr += 1
                        nop = mybir.InstNoOp(
                            name=f"I-wsplit-{ctr}",
                            engine=inst.engine,
                            ins=[],
                            outs=[],
                            sync_info=mybir.SyncInfo(on_wait=[w], on_update=[]),
                        )
                        new_insts.append(nop)
                    si.on_wait = [waits[-1]]
                    changed = True
                new_insts.append(inst)
            if changed:
                bb.instructions[:] = new_insts
    return ctr


def _declare(nc):
    import concourse.mybir as mybir

    bf = mybir.dt.bfloat16
    f8 = mybir.dt.float8e4
    f32 = mybir.dt.float32
    return dict(
        xT8=nc.dram_tensor("xT8", [C, T], f8, kind="ExternalInput").ap(),
        xTb=nc.dram_tensor("xTb", [C, 512], bf, kind="ExternalInput").ap(),
        wqk8=nc.dram_tensor("wqk8", [C, 2 * CQ], f8, kind="ExternalInput").ap(),
        wqkb=nc.dram_tensor("wqkb", [C, 2 * CQ], bf, kind="ExternalInput").ap(),
        wv8=nc.dram_tensor("wv8", [C, CQ], f8, kind="ExternalInput").ap(),
        wvb=nc.dram_tensor("wvb", [C, CQ], bf, kind="ExternalInput").ap(),
        bqk1=nc.dram_tensor("bqk1", [P, 8], f32, kind="ExternalInput").ap(),
        bqk256=nc.dram_tensor("bqk256", [P, 8], f32, kind="ExternalInput").ap(),
        bvb8=nc.dram_tensor("bvb8", [P, CQ], f32, kind="ExternalInput").ap(),
        trimask=nc.dram_tensor("trimask", [P, P], bf, kind="ExternalInput").ap(),
        ident=nc.dram_tensor("ident", [P, P], bf, kind="ExternalInput").ap(),
        identf=nc.dram_tensor("identf", [P, P], mybir.dt.float32,
                              kind="ExternalInput").ap(),
        selp=nc.dram_tensor("selp", [8, 512], bf, kind="ExternalInput").ap(),
        wo=nc.dram_tensor("wo", [CQ, C], bf, kind="ExternalInput").ap(),
        outp=nc.dram_tensor("outp", [T, C], bf, kind="ExternalOutput").ap(),
    )


def _emit(nc, tc, aps):
    import concourse.mybir as mybir
    from concourse.alu_op_type import AluOpType

    bf = mybir.dt.bfloat16
    f8 = mybir.dt.float8e4
    f32 = mybir.dt.float32
    Exp = mybir.ActivationFunctionType.Exp
    DR = mybir.MatmulPerfMode.DoubleRow

    xT8 = aps["xT8"]; xTb = aps["xTb"]; wqk8 = aps["wqk8"]; wqkb = aps["wqkb"]
    wv8 = aps["wv8"]; wvb = aps["wvb"]; bqk1 = aps["bqk1"]
    bqk256 = aps["bqk256"]; bvb8 = aps["bvb8"]; trimask = aps["trimask"]
    ident = aps["ident"]; identf = aps["identf"]; selp = aps["selp"]
    wo = aps["wo"]; outp = aps["outp"]

    with tc.tile_pool(name="const", bufs=1) as cpool:
        bqk1_sb = cpool.tile([P, 8], f32)
        bqk256_sb = cpool.tile([P, 8], f32)
        bvb8_sb = cpool.tile([P, CQ], f32)
        trimask_sb = cpool.tile([P, P], bf)
        ident_sb = cpool.tile([P, P], bf)
        identf_sb = cpool.tile([P, P], f32)
        selp_sb = cpool.tile([8, 512], bf)
        negblk_sb = cpool.tile([P, P], bf)
        ones11_sb = cpool.tile([65, 1], f32)
        xTb_sb = cpool.tile([P, 8 * 512], bf)
        wqkb_sb = cpool.tile([P, 8 * 1024], bf)
        xT8_sb = cpool.tile([P, 8 * T], f8)
        wqk8_sb = cpool.tile([P, 8 * 1024], f8)
        wvb_sb = cpool.tile([P, 8 * CQ], bf)
        wv8_sb = cpool.tile([P, 8 * CQ], f8)
        wo_sb = cpool.tile([P, 4 * 1024], bf)
        qkT_sb = cpool.tile([P, 8 * T], bf)
        v8_sb = cpool.tile([P, 16 * VW8], f8)
        vb16_sb = cpool.tile([P, 2 * VW], bf)
        attn_sb = cpool.tile([P, 16 * 512], bf)

        # DMAs in consumption order: island W/x first (prologue is bf16
        # tt=0 projections), small constants, island wv, then the fp8
        # tensors, later xT8 slices, and wo (only needed by stage 3).
        for cc in range(8):
            nc.sync.dma_start(wqkb_sb[:, cc * 1024:(cc + 1) * 1024],
                              wqkb[cc * P:(cc + 1) * P, :])
            nc.sync.dma_start(xTb_sb[:, cc * 512:(cc + 1) * 512],
                              xTb[cc * P:(cc + 1) * P, :])
        nc.sync.dma_start(bqk1_sb[:], bqk1[:])
        nc.sync.dma_start(bqk256_sb[:], bqk256[:])
        nc.sync.dma_start(trimask_sb[:], trimask[:])
        nc.sync.dma_start(ident_sb[:], ident[:])
        nc.sync.dma_start(identf_sb[:], identf[:])
        nc.sync.dma_start(selp_sb[:], selp[:])
        nc.sync.dma_start(bvb8_sb[:], bvb8[:])
        for cc in range(8):
            nc.sync.dma_start(wvb_sb[:, cc * CQ:(cc + 1) * CQ],
                              wvb[cc * P:(cc + 1) * P, :])
        for cc in range(8):
            nc.sync.dma_start(wv8_sb[:, cc * CQ:(cc + 1) * CQ],
                              wv8[cc * P:(cc + 1) * P, :])
        for cc in range(8):
            nc.sync.dma_start(xT8_sb[:, cc * T: cc * T + 512],
                              xT8[cc * P:(cc + 1) * P, 0:512])
        for cc in range(8):
            nc.sync.dma_start(wqk8_sb[:, cc * 1024:(cc + 1) * 1024],
                              wqk8[cc * P:(cc + 1) * P, :])
        for cc in range(8):
            nc.sync.dma_start(xT8_sb[:, cc * T + 512: cc * T + 1024],
                              xT8[cc * P:(cc + 1) * P, 512:1024])
        for cc in range(8):
            nc.sync.dma_start(xT8_sb[:, cc * T + 1024: (cc + 1) * T],
                              xT8[cc * P:(cc + 1) * P, 1024:T])
        for hc in range(4):
            nc.sync.dma_start(wo_sb[:, hc * 1024:(hc + 1) * 1024],
                              wo[hc * P:(hc + 1) * P, :])

        nc.vector.memset(ones11_sb[:], 1.0)
        nc.gpsimd.memset(negblk_sb[:], -10000.0)
        v8w = v8_sb.rearrange("p (c h e) -> p c h e", c=16, e=128)
        nc.gpsimd.memset(v8w[:, :, :, 64:65], 1.0)    # denominator ones
        nc.gpsimd.memset(v8w[:, :, :, 65:128], 0.0)   # pad cols
        vb16w = vb16_sb.rearrange("p (c h e) -> p c h e", c=2, e=65)
        nc.gpsimd.memset(vb16w[:, :, :, 64:65], 1.0)

        # fp8 operand views with the DoubleRow "two" dim = contraction
        # chunk pairs (cc, cc+1)
        xT8v = xT8_sb.rearrange("p (cc t) -> p cc t", cc=8)
        wqk8v = wqk8_sb.rearrange("p (cc n) -> p cc n", cc=8)
        wv8v = wv8_sb.rearrange("p (cc n) -> p cc n", cc=8)
        v8v = v8_sb.rearrange("p (c w) -> p c w", c=16)
        bvb8v = bvb8_sb.rearrange("p (a h e) -> p a h e", a=1, e=64)

        with tc.tile_pool(name="ps_s", bufs=1, space="PSUM") as ps_s, \
             tc.tile_pool(name="ps_av", bufs=1, space="PSUM") as ps_av, \
             tc.tile_pool(name="ps_aux", bufs=1, space="PSUM") as ps_aux, \
             tc.tile_pool(name="work", bufs=1) as wpool:

            pair_tiles = {}
            recs = {}

            # ---------- pipeline building blocks ----------
            def proj_qk(nt, tt):
                # qkT feature tile nt (q: nt=hp, k: nt=4+hp), 512 tokens.
                # tt=0 is the bf16 island; tt>=1 runs fp8 DoubleRow.
                psq = ps_aux.tile([P, 512], f32, tag="aux", bufs=2,
                                  name=f"psq_{nt}_{tt}")
                sl = slice(nt * T + tt * 512, nt * T + (tt + 1) * 512)
                if tt == 0:
                    for cc in range(8):
                        nc.tensor.matmul(
                            psq[:],
                            wqkb_sb[:, cc * 1024 + nt * P: cc * 1024 + (nt + 1) * P],
                            xTb_sb[:, cc * 512:(cc + 1) * 512],
                            start=(cc == 0), stop=(cc == 7),
                        )
                    nc.vector.tensor_scalar(
                        qkT_sb[:, sl], psq[:], bqk1_sb[:, nt:nt + 1], None,
                        op0=AluOpType.add,
                    )
                else:
                    for cp in range(4):
                        nc.tensor.matmul(
                            psq[:],
                            wqk8v[:, 2 * cp:2 * cp + 2, nt * P:(nt + 1) * P],
                            xT8v[:, 2 * cp:2 * cp + 2, tt * 512:(tt + 1) * 512],
                            start=(cp == 0), stop=(cp == 3), perf_mode=DR,
                        )
                    nc.vector.tensor_scalar(
                        qkT_sb[:, sl], psq[:], bqk256_sb[:, nt:nt + 1],
                        1.0 / 256.0, op0=AluOpType.add, op1=AluOpType.mult,
                    )

            def proj_v(t16):
                # v token tile t16 (128 tokens). t16<2 is the bf16 island
                # (also stored fp8); t16>=2 runs fp8 DoubleRow.
                psv = ps_aux.tile([P, CQ], f32, tag="aux", bufs=2,
                                  name=f"psv_{t16}")
                v8sl = v8v[:, t16:t16 + 1, :].rearrange(
                    "p a (h e) -> p a h e", e=128)[:, :, :, 0:64]
                psvv = psv.rearrange("p (a h e) -> p a h e", a=1, e=64)
                if t16 < 2:
                    for cc in range(8):
                        nc.tensor.matmul(
                            psv[:],
                            xTb_sb[:, cc * 512 + t16 * P: cc * 512 + (t16 + 1) * P],
                            wvb_sb[:, cc * CQ:(cc + 1) * CQ],
                            start=(cc == 0), stop=(cc == 7),
                        )
                    vbsl = vb16w[:, t16:t16 + 1, :, 0:64]
                    nc.vector.scalar_tensor_tensor(
                        vbsl, psvv, 8.0, bvb8v, op0=AluOpType.mult,
                        op1=AluOpType.add,
                    )
                    nc.vector.tensor_copy(v8sl, vbsl)
                else:
                    for cp in range(4):
                        nc.tensor.matmul(
                            psv[:],
                            xT8v[:, 2 * cp:2 * cp + 2, t16 * P:(t16 + 1) * P],
                            wv8v[:, 2 * cp:2 * cp + 2, :],
                            start=(cp == 0), stop=(cp == 3), perf_mode=DR,
                        )
                    nc.vector.scalar_tensor_tensor(
                        v8sl, psvv, 1.0 / 32.0, bvb8v, op0=AluOpType.mult,
                        op1=AluOpType.add,
                    )

            def s_chunk(hp, tb, j, pt, c):
                # S^T chunk for key block j, queries tb*512:(tb+1)*512, both
                # heads of pair hp. Masks applied on the PE as -1e4 adds.
                h0, h1 = 2 * hp, 2 * hp + 1
                off = j * P - tb * 512
                nstart = max(off, 0)
                strip = (c == 1 and off >= 128)
                estart = off - 128 if strip else nstart
                pss = ps_s.tile([P, 1024], f32, tag="pss", bufs=2,
                                name=f"pss_{hp}_{tb}_{j}")
                for i, hl in enumerate((h0, h1)):
                    base = (hl % 2) * 64
                    diag = off >= 0
                    nc.tensor.matmul(
                        pss[:, i * 512 + nstart: i * 512 + 512],
                        qkT_sb[base:base + 64,
                               (4 + hp) * T + j * P: (4 + hp) * T + (j + 1) * P],
                        qkT_sb[base:base + 64,
                               hp * T + tb * 512 + nstart: hp * T + (tb + 1) * 512],
                        start=True, stop=not diag,
                    )
                    if diag:
                        if strip:
                            nc.tensor.matmul(
                                pss[:, i * 512 + estart: i * 512 + estart + P],
                                ident_sb[:], negblk_sb[:],
                                start=False, stop=False, skip_group_check=True,
                            )
                        nc.tensor.matmul(
                            pss[:, i * 512 + off: i * 512 + off + P],
                            ident_sb[:], trimask_sb[:],
                            start=False, stop=True, skip_group_check=True,
                        )
                psw = pss.rearrange("p (h q) -> p h q", q=512)
                ptw = pt.rearrange("p (ch q) -> p ch q", q=512)
                nc.scalar.activation(
                    ptw[:, 2 * c:2 * c + 2, estart:512],
                    psw[:, :, estart:512], Exp, scale=0.125,
                )

            def s_pair(hp, tb, m):
                island = (tb == 0 and m == 0)
                if island:
                    pt = wpool.tile([P, 2048], bf, tag="ptb", bufs=2,
                                    name=f"ptb_{hp}")
                else:
                    pt = wpool.tile([P, 2048], f8, tag="pt", bufs=10,
                                    name=f"pt_{hp}_{tb}_{m}")
                pair_tiles[(hp, tb, m)] = (pt, island)
                s_chunk(hp, tb, 2 * m, pt, 0)
                s_chunk(hp, tb, 2 * m + 1, pt, 1)

            def av_pair(hp, tb, m, psav):
                h0 = 2 * hp
                last = 2 * tb + 1
                de = max(2 * m * P - tb * 512, 0)
                pt, island = pair_tiles.pop((hp, tb, m))
                ptw = pt.rearrange("p (ch q) -> p ch q", q=512)
                pt4 = pt.rearrange("p (c i q) -> p c i q", c=2, q=512)
                for i in range(2):
                    h = h0 + i
                    if island:
                        for c in range(2):
                            nc.tensor.matmul(
                                psav[i][0:65, :],
                                vb16w[:, c:c + 1, h:h + 1, :],
                                ptw[:, 2 * c + i:2 * c + i + 1, :],
                                start=(c == 0), stop=False,
                                skip_group_check=True,
                            )
                    else:
                        nc.tensor.matmul(
                            psav[i][:, de:512],
                            v8v[:, 2 * m:2 * m + 2, h * 128:(h + 1) * 128],
                            pt4[:, :, i:i + 1, de:512],
                            start=(m == 0), stop=(m == last), perf_mode=DR,
                            skip_group_check=True,
                        )
                if m == last:
                    seg = hp * NTT + tb
                    sl = slice(seg * 512, (seg + 1) * 512)
                    # reciprocal() is ~6.7 ns/elem PER LANE; a [1,512] row
                    # costs 3.4us. Spread the 1024 denominators across the
                    # 128 partitions first (PE transposes), reciprocal the
                    # [128,8] column block (~0.3us), transpose back.
                    drow = wpool.tile([65, 1024], f32, tag="drow", bufs=2,
                                      name=f"drow_{seg}")
                    nc.vector.tensor_copy(drow[64:65, 0:512],
                                          psav[0][64:65, :])
                    nc.vector.tensor_copy(drow[64:65, 512:1024],
                                          psav[1][64:65, :])
                    dps = ps_aux.tile([P, 512], f32, tag="aux", bufs=2,
                                      name=f"dps_{seg}")
                    for k in range(8):
                        nc.tensor.matmul(dps[:, k:k + 1],
                                         drow[64:65, k * P:(k + 1) * P],
                                         ones11_sb[64:65, :],
                                         start=True, stop=True)
                    rcol = wpool.tile([P, 8], f32, tag="rcol", bufs=2,
                                      name=f"rcol_{seg}")
                    with nc.allow_low_precision(reason="bf16 softmax denom"):
                        nc.vector.reciprocal(rcol[:], dps[:, 0:8])
                    nc.tensor.matmul(dps[0:8, 256:384], rcol[:], identf_sb[:],
                                     is_transpose=True, start=True, stop=True,
                                     skip_group_check=True)
                    rrow = wpool.tile([8, 128], bf, tag="rrow", bufs=3,
                                      name=f"rrow_{seg}")
                    with nc.allow_low_precision(reason="bf16 softmax denom"):
                        nc.vector.tensor_copy(rrow[:], dps[0:8, 256:384])
                    recs[seg] = rrow
                    nc.vector.tensor_copy(attn_sb[0:64, sl], psav[0][0:64, :])
                    nc.vector.tensor_copy(attn_sb[64:128, sl], psav[1][0:64, :])

            def norm_seg(seg):
                sl = slice(seg * 512, (seg + 1) * 512)
                rrow = recs.pop(seg)
                psr = ps_aux.tile([P, 512], f32, tag="aux", bufs=2,
                                  name=f"psr_{seg}")
                # psr[64i:64i+64, 128b:128b+128] = 0.125 * rrow[4i+b, :]
                # via selector matmuls (selp row j = 0.125 one-hot)
                for j in range(8):
                    i, blk = j // 4, j % 4
                    nc.tensor.matmul(
                        psr[64 * i:64 * i + 64, blk * P:(blk + 1) * P],
                        selp_sb[:, j * 64:(j + 1) * 64], rrow[:],
                        start=True, stop=True, skip_group_check=True,
                    )
                nc.vector.tensor_tensor(attn_sb[:, sl], attn_sb[:, sl], psr[:],
                                        op=AluOpType.mult)

            def oproj_tt(tt16, pspool=None, ptag="aux"):
                psos = [
                    (pspool or ps_aux).tile([P, 512], f32, tag=ptag, bufs=2,
                                            name=f"pso_{tt16}_{mb}")
                    for mb in range(2)
                ]
                for hc in range(4):
                    seg = hc * NTT + tt16 // 4
                    col = (seg * 4 + tt16 % 4) * P
                    for mb in range(2):
                        nc.tensor.matmul(
                            psos[mb][:],
                            attn_sb[:, col: col + P],
                            wo_sb[:, hc * 1024 + mb * 512: hc * 1024 + (mb + 1) * 512],
                            start=(hc == 0), stop=(hc == 3),
                        )
                for mb in range(2):
                    osb = wpool.tile([P, 512], bf, tag="osb", bufs=4,
                                     name=f"osb_{tt16}_{mb}")
                    nc.vector.tensor_copy(osb[:], psos[mb][:])
                    nc.sync.dma_start(
                        outp[tt16 * P:(tt16 + 1) * P, mb * 512:(mb + 1) * 512],
                        osb[:],
                    )

            # ---------- per-segment filler schedules ----------
            # Filler emitted during segment s must only feed emissions of
            # segment s+1 or later (PE queue is in-order).
            def fillers_for(s):
                hp, tb = s // 4, s % 4
                out = []
                if hp == 0:
                    if tb < 2:
                        out.append(lambda tt=tb + 2: proj_qk(0, tt))
                        out.append(lambda tt=tb + 2: proj_qk(4, tt))
                    if tb < 3:
                        for i in range(4):
                            out.append(lambda t=4 * (tb + 1) + i: proj_v(t))
                if hp < 3:
                    out.append(lambda h=hp + 1, tt=tb: proj_qk(h, tt))
                    out.append(lambda h=hp + 1, tt=tb: proj_qk(4 + h, tt))
                if hp == 3 and tb >= 1:
                    for i in range(4):
                        out.append(lambda t=4 * (tb - 1) + i: oproj_tt(t))
                return out

            # ---------- prologue ----------
            proj_qk(0, 0)   # bf16 island: q features of head pair 0
            proj_qk(4, 0)   # bf16 island: k features
            s_pair(0, 0, 0)  # spool up ACT as early as possible
            s_pair(0, 0, 1)
            for t16 in range(4):
                proj_v(t16)
            proj_qk(0, 1)
            proj_qk(4, 1)

            # ---------- main pipeline ----------
            for s in range(16):
                hp, tb = s // 4, s % 4
                psav = [
                    ps_av.tile([P, 512], f32, tag="psav", bufs=2,
                               name=f"psav_{hp}_{tb}_{i}")
                    for i in range(2)
                ]
                npa = 2 * tb + 2
                if s + 1 < 16:
                    nhp, ntb = (s + 1) // 4, (s + 1) % 4
                    nps = 2 * ntb + 2
                else:
                    nps = 0
                fill = fillers_for(s)
                nsteps = max(npa, nps)
                for m in range(nsteps):
                    if s > 0 and m == 1:
                        norm_seg(s - 1)
                    if m < nps:
                        s_pair(nhp, ntb, m)
                    if m < npa:
                        av_pair(hp, tb, m, psav)
                    while fill and len(fill) >= (nsteps - m):
                        fill.pop(0)()
                while fill:
                    fill.pop(0)()

            # ---------- epilogue ----------
            norm_seg(15)
            for t16 in range(12, 16):
                oproj_tt(t16, pspool=ps_av, ptag="psav")


_cached = {}


def build_program(split=True, ncopies=1):
    key = ("nc", ncopies)
    if key not in _cached:
        import concourse.bass as bass
        import concourse.tile as tile

        nc = bass.Bass("TRN2", target_bir_lowering=False, debug=False)
        with tile.TileContext(nc) as tc:
            aps = _declare(nc)
            for _ in range(ncopies):
                _emit(nc, tc, aps)
        _cached[key] = nc
    if split and not _cached.get(("split", ncopies)):
        _split_waits(_cached[key])
        _cached[("split", ncopies)] = True
    return _cached[key]


def make_in_maps(x, W_qkv, b_qkv, W_o):
    x = np.asarray(x, dtype=np.float32)
    W_qkv = np.asarray(W_qkv, dtype=np.float32)
    b_qkv = np.asarray(b_qkv, dtype=np.float32)
    W_o = np.asarray(W_o, dtype=np.float32)
    ar = np.arange(P)
    trimaskN = np.where(ar[:, None] > ar[None, :], -10000.0, 0.0).astype(BF16)
    identity = np.eye(P, dtype=np.float32).astype(BF16)
    identityf = np.eye(P, dtype=np.float32)
    selp = np.zeros((8, 512), np.float32)
    for j in range(8):
        selp[j, j * 64:(j + 1) * 64] = 0.125
    selp = selp.astype(BF16)
    in_maps = []
    for core in range(NCORES):
        b, g = core // 2, core % 2
        qs = slice(g * CQ, (g + 1) * CQ)
        xT = np.ascontiguousarray(x[b].T)
        wq = W_qkv[:, 0:C][:, qs]
        wk = W_qkv[:, C:2 * C][:, qs]
        wvs = np.ascontiguousarray(W_qkv[:, 2 * C:3 * C][:, qs])
        wqks = np.ascontiguousarray(np.concatenate([wq, wk], axis=1))
        bq = b_qkv[0:C][qs]
        bk = b_qkv[C:2 * C][qs]
        bv = b_qkv[2 * C:3 * C][qs]
        bqk_t = np.ascontiguousarray(
            np.concatenate([bq, bk]).reshape(8, P).T
        ).astype(np.float32)
        bvb8 = np.ascontiguousarray(
            np.broadcast_to(8.0 * bv, (P, CQ))
        ).astype(np.float32)
        wos = np.ascontiguousarray(W_o[qs, :]).astype(BF16)
        in_maps.append(
            dict(
                xT8=(xT * 4.0).astype(F8E4),
                xTb=np.ascontiguousarray(xT[:, 0:512]).astype(BF16),
                wqk8=(wqks * 64.0).astype(F8E4),
                wqkb=wqks.astype(BF16),
                wv8=(wvs * 64.0).astype(F8E4),
                wvb=wvs.astype(BF16),
                bqk1=bqk_t,
                bqk256=(bqk_t * 256.0).astype(np.float32),
                bvb8=bvb8,
                trimask=trimaskN,
                ident=identity,
                identf=identityf,
                selp=selp,
                wo=wos,
            )
        )
    return in_maps


def run(x, W_qkv, b_qkv, W_o, b_o, trace=False, trace_kwargs=None):
    import time as _time

    from concourse.bass_utils import run_bass_kernel_spmd

    nc = build_program()
    in_maps = make_in_maps(x, W_qkv, b_qkv, W_o)
    last_err = None
    for attempt in range(3):
        try:
            res = run_bass_kernel_spmd(
                nc, in_maps, core_ids=list(range(NCORES)), trace=trace,
                **(trace_kwargs or {}),
            )
            break
        except Exception as e:  # transient device wedge -> retry
            last_err = e
            _time.sleep(5)
    else:
        raise last_err
    b_o = np.asarray(b_o, dtype=np.float32)
    out = np.empty((B, T, C), np.float32)
    for b in range(B):
        out[b] = (res.results[2 * b]["outp"].astype(np.float32)
                  + res.results[2 * b + 1]["outp"].astype(np.float32) + b_o)
    return out, res


def kernel(x, W_qkv, b_qkv, W_o, b_o):
    out, _ = run(x, W_qkv, b_qkv, W_o, b_o, trace=False)
    return out
